# revision 1
# baseline (speedup 1.0000x reference)
"""BasicNCA (neural cellular automaton) Trainium2 kernel, 8-core SPMD.

Reference computation (per step, 32 steps):
  p  = depthwise3x3(s, [identity, sobel_x, sobel_y])   # (B, 3C, H, W)
  h  = relu(w1 @ p + b1)                               # (B, 64, H, W)
  d  = w2 @ h + b2                                     # (B, C, H, W)
  s += d * (mask < 0.5)

Implementation notes:
- The perception conv + first 1x1 conv fuse into one effective 3x3 conv with
  weights Weff[o, c, dy, dx]; computed on the PE as 9 shifted "tap" matmuls
  accumulating in PSUM (fp32r operands, full rate at free dim >= 256).
- Sharding: core i handles batch i//2, H-half i%2, with a 32-row taper of
  redundant compute instead of per-step halo exchange between cores
  (validity shrinks 1 row/step; 32 steps consume exactly the margin).
- A core's 96-row slab splits into 4 sub-slabs of 24 rows on the 4 SBUF
  partition quadrants (channels 0-23 of each), so DVE/ACT elementwise work
  runs ~120 lanes wide.
- This walrus build rejects column tile positions != 0 and crashes on PSUM
  accumulation chains that change tile_position mid-chain. So: taps process
  slab PAIRS with K=56 block lhsT (both slabs' h stacked in the output's
  partition halves, one tile_position per chain); the second 1x1 conv runs
  in full 128x128 mode (K=128 over both slabs' h, M=120 with per-slab
  24-column blocks so deltas land on the right partition quadrants).
- Masks are pre-replicated across channel lanes on the host and streamed per
  step (DMA is otherwise idle).
"""

import sys

sys.path.insert(0, "/opt/trn_rl_repo")

import numpy as np

import concourse.bass as bass
import concourse.bacc as bacc
import concourse.tile as tile
import concourse.mybir as mybir

dt = mybir.dt

B, C, H, W = 4, 24, 128, 128
HID = 64
FIRE_RATE = 0.5
N_CORES = 8

SH = 96            # slab rows per core (64 own + 32 taper)
SR = 24            # rows per sub-slab (one partition quadrant)
FW = W + 2         # padded row width (130)
FR = SR + 2        # frame rows per sub-slab (26)
FRAME_OFF = 4      # leading guard elems so tap offset -1 stays in-bounds
FRAME = FR * FW    # 3380
S_FREE = FRAME_OFF + FRAME + 4
COMP = SR * FW     # 3120 compact free size (real rows 0..23)
NCH = 390          # chunk = 3 rows
NCHUNK = COMP // NCH  # 8

LAST_EXEC_NS = None
_cache = {}


def _taps():
    # correlation taps: out(y, x) = sum_{dy,dx} in(y+dy-1, x+dx-1) * k[dy, dx]
    return [(dy, dx) for dy in range(3) for dx in range(3)]


def _build_program(steps, apply_b2, repeats=1):
    nc = bacc.Bacc("TRN2", target_bir_lowering=False, debug=False,
                   num_devices=N_CORES)

    s_d = nc.dram_tensor("s0", [128, S_FREE], dt.float32r, kind="ExternalInput")
    m_d = nc.dram_tensor("masks", [steps, 128, COMP], dt.float32,
                         kind="ExternalInput")
    tapw_d = nc.dram_tensor("tapw", [128, 9 * 128], dt.float32r,
                            kind="ExternalInput")
    w2b_d = nc.dram_tensor("w2b", [128, 2 * 120], dt.float32r,
                           kind="ExternalInput")
    b2r_d = nc.dram_tensor("b2r", [128, 1], dt.float32, kind="ExternalInput")
    b1_d = nc.dram_tensor("b1v", [128, 1], dt.float32, kind="ExternalInput")
    out_d = nc.dram_tensor("out", [128, SR * W], dt.float32,
                           kind="ExternalOutput")

    with tile.TileContext(nc) as tc:
        import os as _os
        _mb = int(_os.environ.get("NCA_MB", "2"))
        _hb = int(_os.environ.get("NCA_HB", "3"))
        _ub = int(_os.environ.get("NCA_UB", "2"))
        with tc.tile_pool(name="persist", bufs=1) as pp, \
             tc.tile_pool(name="mpool", bufs=_mb) as mpool, \
             tc.tile_pool(name="hsb", bufs=_hb) as hsbp, \
             tc.tile_pool(name="upool", bufs=_ub) as upool, \
             tc.tile_pool(name="hps", bufs=int(__import__("os").environ.get("NCA_HPSB", "3")), space="PSUM") as hps_pool, \
             tc.tile_pool(name="dps", bufs=int(__import__("os").environ.get("NCA_DPSB", "1")), space="PSUM") as dps_pool:

            s_sb = pp.tile([128, S_FREE], dt.float32r)
            tapw = pp.tile([128, 9 * 128], dt.float32r)
            w2b = pp.tile([128, 2 * 120], dt.float32r)
            b2r = pp.tile([128, 1], dt.float32)
            b1v = pp.tile([128, 1], dt.float32)

            nc.sync.dma_start(s_sb[:], s_d[:])
            nc.sync.dma_start(tapw[:], tapw_d[:])
            nc.sync.dma_start(w2b[:], w2b_d[:])
            nc.sync.dma_start(b2r[:], b2r_d[:])
            nc.sync.dma_start(b1v[:], b1_d[:])

            taps = _taps()

            for t in range(steps * repeats):
                t = t % steps
                m_sb = mpool.tile([128, COMP], dt.float32, tag="m")
                nc.sync.dma_start(m_sb[:], m_d[t])

                u_sb = upool.tile([128, COMP], dt.float32, tag="u")

                # zigzag chunk order: consecutive steps meet at the same
                # edge, shortening the serial step-boundary chain
                korder = (range(NCHUNK // 2) if t % 2 == 0
                          else range(NCHUNK // 2 - 1, -1, -1))
                for k in korder:                    # chunk pairs
                    hsb_list = []
                    for p in range(2):              # slab pairs {0,1} / {2,3}
                        base = 64 * p
                        hps = hps_pool.tile([128, 1024], dt.float32,
                                            tag="hps")
                        for cc in range(2):
                            c = 2 * k + cc
                            for ti, (dy, dx) in enumerate(taps):
                                off = (FRAME_OFF + (3 * c + dy) * FW
                                       + dx - 1)
                                nc.tensor.matmul(
                                    hps[:, 512 * cc:512 * cc + NCH],
                                    tapw[base:base + 56,
                                         128 * ti:128 * ti + 128],
                                    s_sb[base:base + 56, off:off + NCH],
                                    start=(ti == 0), stop=(ti == 8),
                                    tile_position=(base, 0),
                                )
                        # relu + b1 for both chunks of this slab pair
                        hsb = hsbp.tile([128, 2 * NCH], dt.float32r,
                                        tag=f"hsb{p}")
                        hsb_list.append(hsb)
                        nc.scalar.activation(
                            hsb[:].rearrange("p (b x) -> p b x", x=NCH),
                            hps[:].rearrange("p (b x) -> p b x", b=2)[:, :, 0:NCH],
                            mybir.ActivationFunctionType.Relu,
                            bias=b1v[:, 0:1],
                        )

                    dps = dps_pool.tile([128, 1024], dt.float32, tag="dps")
                    for cc in range(2):
                        for p in range(2):
                            nc.tensor.matmul(
                                dps[0:120, 512 * cc:512 * cc + NCH],
                                w2b[:, 120 * p:120 * p + 120],
                                hsb_list[p][:, NCH * cc:NCH * cc + NCH],
                                start=(p == 0), stop=(p == 1),
                            )
                    if apply_b2:
                        # delta += b2 (per-partition scalar), in psum
                        nc.vector.tensor_scalar_add(
                            dps[0:120].rearrange(
                                "p (b x) -> p b x", b=2)[:, :, 0:NCH],
                            dps[0:120].rearrange(
                                "p (b x) -> p b x", b=2)[:, :, 0:NCH],
                            b2r[0:120, 0:1],
                        )

                    # u = (m < 0.5) * delta for this chunk pair
                    nc.vector.scalar_tensor_tensor(
                        u_sb[0:120, 780 * k:780 * k + 780].rearrange(
                            "p (b x) -> p b x", x=NCH),
                        m_sb[0:120, 780 * k:780 * k + 780].rearrange(
                            "p (b x) -> p b x", x=NCH),
                        FIRE_RATE,
                        dps[0:120].rearrange("p (b x) -> p b x", b=2)[:, :, 0:NCH],
                        mybir.AluOpType.is_lt,
                        mybir.AluOpType.mult,
                    )

                # s += u, split per chunk-pair so it pipelines with later
                # chunks' taps (the tap reads of neighboring rows gate each
                # piece via Tile's range tracking)
                for k in korder:
                    a = FRAME_OFF + FW + 780 * k
                    nc.vector.tensor_add(
                        s_sb[0:120, a:a + 780],
                        s_sb[0:120, a:a + 780],
                        u_sb[0:120, 780 * k:780 * k + 780],
                    )

                # intra-core halo refresh between sub-slabs
                if True:
                    for g in range(3):
                        nc.sync.dma_start(
                            s_sb[32 * g:32 * g + 24,
                                 FRAME_OFF + 25 * FW:FRAME_OFF + 25 * FW + FW],
                            s_sb[32 * (g + 1):32 * (g + 1) + 24,
                                 FRAME_OFF + FW:FRAME_OFF + FW + FW],
                        )
                        nc.sync.dma_start(
                            s_sb[32 * (g + 1):32 * (g + 1) + 24,
                                 FRAME_OFF:FRAME_OFF + FW],
                            s_sb[32 * g:32 * g + 24,
                                 FRAME_OFF + 24 * FW:FRAME_OFF + 24 * FW + FW],
                        )

            # write back real pixels (frame rows 1..24, cols 1..128)
            a0 = FRAME_OFF + FW + 1
            nc.sync.dma_start(
                out_d[:].rearrange("p (r x) -> p r x", x=W),
                s_sb[:, a0:a0 + SR * FW].rearrange(
                    "p (r x) -> p r x", x=FW)[:, :, 0:W].bitcast(dt.float32),
            )

    nc.compile()
    return nc


def _prep_weights(w1, b1, w2, b2):
    sx = np.array([[-1, 0, 1], [-2, 0, 2], [-1, 0, 1]], np.float32) / 8.0
    sy = sx.T.copy()
    ident = np.zeros((3, 3), np.float32)
    ident[1, 1] = 1.0
    # Weff[o, c, dy, dx]
    weff = (np.einsum("oc,yx->ocyx", w1[:, 0::3], ident)
            + np.einsum("oc,yx->ocyx", w1[:, 1::3], sx)
            + np.einsum("oc,yx->ocyx", w1[:, 2::3], sy)).astype(np.float32)

    # pair-tap lhsT: K=56 rows (quadrants q, q+1 channels), M=128
    # rows 0-23 -> h of even slab at out partitions 0-63,
    # rows 32-55 -> h of odd slab at out partitions 64-127.
    tapw = np.zeros((128, 9 * 128), np.float32)
    for ti, (dy, dx) in enumerate(_taps()):
        wt = weff[:, :, dy, dx].T          # [24, 64]
        for p in range(2):
            base = 64 * p
            tapw[base:base + 24, 128 * ti:128 * ti + 64] = wt
            tapw[base + 32:base + 56, 128 * ti + 64:128 * ti + 128] = wt

    # layer2 lhsT per pair: K=128 (both h halves), M=120 with 24-col blocks
    # placing each slab's delta on its partition quadrant.
    w2b = np.zeros((128, 2 * 120), np.float32)
    for p in range(2):
        ge, go = 2 * p, 2 * p + 1
        w2b[0:64, 120 * p + 32 * ge:120 * p + 32 * ge + 24] = w2.T
        w2b[64:128, 120 * p + 32 * go:120 * p + 32 * go + 24] = w2.T

    b2r = np.zeros((128, 1), np.float32)
    b1v = np.zeros((128, 1), np.float32)
    for g in range(4):
        b2r[32 * g:32 * g + 24, 0] = b2
    b1v[0:64, 0] = b1
    b1v[64:128, 0] = b1
    return tapw, w2b, b2r, b1v


def _prep_state(state):
    """state (B, C, H, W) -> per-core [128, S_FREE] framed slabs."""
    bufs = []
    for core in range(N_CORES):
        b = core // 2
        top = (core % 2) == 0
        r0 = 0 if top else H - SH
        buf = np.zeros((128, S_FREE), np.float32)
        for ch in range(C):
            full = np.zeros((SH + 2, FW), np.float32)
            full[1:SH + 1, 1:W + 1] = state[b, ch, r0:r0 + SH, :]
            if r0 > 0:
                full[0, 1:W + 1] = state[b, ch, r0 - 1, :]
            if r0 + SH < H:
                full[SH + 1, 1:W + 1] = state[b, ch, r0 + SH, :]
            for g in range(4):
                fr = full[g * SR:g * SR + FR, :]
                buf[32 * g + ch, FRAME_OFF:FRAME_OFF + FRAME] = fr.reshape(-1)
        bufs.append(buf)
    return bufs


def _prep_masks(masks):
    """masks (S, B, 1, H, W) -> per-core [S, 128, COMP] fire-padded."""
    S = masks.shape[0]
    bufs = []
    for core in range(N_CORES):
        b = core // 2
        top = (core % 2) == 0
        r0 = 0 if top else H - SH
        mb = np.ones((S, 128, COMP), np.float32)
        mrows = np.ones((S, SH, FW), np.float32)
        mrows[:, :, 1:W + 1] = masks[:, b, 0, r0:r0 + SH, :]
        for g in range(4):
            seg = mrows[:, g * SR:(g + 1) * SR, :].reshape(S, COMP)
            mb[:, 32 * g:32 * g + C, :] = seg[:, None, :]
        bufs.append(mb)
    return bufs


def kernel(state, w1, b1, w2, b2, masks):
    state = np.asarray(state)
    w1, b1 = np.asarray(w1), np.asarray(b1)
    w2, b2 = np.asarray(w2), np.asarray(b2)
    masks = np.asarray(masks)
    import os as _os
    steps = masks.shape[0]
    apply_b2 = bool(np.any(b2 != 0))
    repeats = int(_os.environ.get("NCA_REPEAT", "1"))
    key = ("prog", steps, apply_b2, repeats)
    if key not in _cache:
        _cache[key] = _build_program(steps, apply_b2, repeats)
    nc = _cache[key]

    from concourse.bass_utils import run_bass_kernel_spmd

    tapw, w2b, b2r, b1v = _prep_weights(w1, b1, w2, b2)
    s_bufs = _prep_state(state)
    m_bufs = _prep_masks(masks)

    in_maps = []
    for core in range(N_CORES):
        in_maps.append({
            "s0": s_bufs[core],
            "masks": m_bufs[core],
            "tapw": tapw,
            "w2b": w2b,
            "b2r": b2r,
            "b1v": b1v,
        })

    import os
    trace = bool(os.environ.get("NCA_TRACE"))
    kw = {}
    if trace:
        kw["trace"] = True
        if os.environ.get("NCA_TRACE_DIR"):
            kw["tmpdir"] = os.environ["NCA_TRACE_DIR"]
    res = run_bass_kernel_spmd(nc, in_maps, list(range(N_CORES)), **kw)
    global LAST_EXEC_NS
    LAST_EXEC_NS = res.exec_time_ns

    out = np.zeros((B, C, H, W), np.float32)
    for core in range(N_CORES):
        o = res.results[core]["out"]  # [128, SR*W]
        b = core // 2
        top = (core % 2) == 0
        r0 = 0 if top else H - SH
        own0 = 0 if top else H // 2
        for g in range(4):
            rows = o[32 * g:32 * g + 24].reshape(C, SR, W)
            g0 = r0 + g * SR
            lo = max(g0, own0)
            hi = min(g0 + SR, own0 + H // 2)
            if lo < hi:
                out[b, :, lo:hi, :] = rows[:, lo - g0:hi - g0, :]
    return out



# revision 23
# speedup vs baseline: 1.6697x; 1.6697x over previous
"""BasicNCA (neural cellular automaton) Trainium2 kernel, 8-core SPMD, v2.

Reference computation (per step, 32 steps):
  p  = depthwise3x3(s, [identity, sobel_x, sobel_y])   # (B, 3C, H, W)
  h  = relu(w1 @ p + b1)                               # (B, 64, H, W)
  d  = w2 @ h + b2                                     # (B, C, H, W)
  s += d * (mask < 0.5)

v2 strategy (vs the fp32r 9-tap baseline):
- fp8e4m3 DoubleRow matmuls: each PE instruction contracts TWO k-tiles
  (weight slots) at 0.5 cycles/output-row, 4x the fp32r tap rate.  The
  separable sobel structure packs the whole perception+w1 layer into 6
  DoubleRow matmuls per chunk (vs 9 full-rate fp32r taps):
    sobel_x = [1,2,1]^T (x) [-1,0,1]/8 -> six +-B/8-weighted s8 windows
    sobel_y = [-1,0,1]^T (x) [1,2,1]/8 -> v8 = s(y+1)-s(y-1) materialized
      on DVE, windows v8@{x-1,x+1} + center expanded to s8 rows +-1
    identity -> A @ s8, plus a hi-lo correction slot A_lo @ (s/16) that
      recovers most of the fp8 weight-quantization error (the identity
      path dominates it; sobel matrices B, C enter /8 so their error is
      small).  Weight slots hold q8(M*WS) and exact power-of-2 scalings;
      the 1/WS unscale folds into the relu's activation scale.
- h is bf16 (relu output); the 1x1 layer-2 matmul runs in bf16 at full
  rate, K=128 over both slabs' h, M=120 as in the baseline.
- fire masks are precomputed on the host as fp8 0/1, quartering the
  per-step DMA stream vs fp32 uniforms.
- Elementwise work is spread across all three non-PE engines: v8 + fire
  mult (+2 relu chunks) on DVE, s8/s8d casts (+4 relu chunks) on ACT,
  s += u adds (+2 relu chunks) on GpSimd.  With b1 == 0 relu is a plain
  scale+max; a nonzero b1 falls back to all-ACT activation relu.
- Sharding unchanged: core i = batch i//2, H-half i%2, 96-row slab with
  a 32-row taper of redundant compute (no cross-core exchange); 4
  sub-slabs of 24 rows on the SBUF partition quadrants; 3-row chunks.
"""

import sys

sys.path.insert(0, "/opt/trn_rl_repo")

import numpy as np

import concourse.bass as bass
import concourse.bacc as bacc
import concourse.tile as tile
import concourse.mybir as mybir
from concourse.ap import AP

dt = mybir.dt

B, C, H, W = 4, 24, 128, 128
HID = 64
FIRE_RATE = 0.5
N_CORES = 8

SH = 96            # slab rows per core (64 own + 32 taper)
SR = 24            # rows per sub-slab (one partition quadrant)
FW = W + 2         # padded row width (130)
FR = SR + 2        # frame rows per sub-slab (26)
FRAME_OFF = 4      # leading guard elems so tap offset -1 stays in-bounds
FRAME = FR * FW    # 3380
S_FREE = FRAME_OFF + FRAME + 4
COMP = SR * FW     # 3120 compact free size (real rows 0..23)
NCH = 390          # chunk = 3 rows
NCHUNK = COMP // NCH  # 8

# fp8 scratch tile: three FRAME-sized regions (s8, v8, s8d) at a common
# pitch so the hi-lo slot's k-tile stride is constant.
G8 = 8
F8T = FRAME + 2 * G8          # 3396 region pitch
S8O = G8                       # s8 = q8(s)
V8O = G8 + F8T                 # v8 = q8(s(y+1) - s(y-1))
SV_TOTAL = 2 * F8T + 2 * G8

WS = 32.0          # weight pre-scale keeping q8(M*WS) in e4m3 normal range
NSLOT = 6          # DoubleRow matmuls per chunk

LAST_EXEC_NS = None
_cache = {}


def _slot_table(c):
    """Per-chunk DoubleRow matmul table: (j0 offset, k-tile stride).

    Window offsets are relative to the sv8 tile; weights live in tapw8
    blocks of 256 (= 2 k-tiles x 128 out) per slot, see _prep_weights.
    """
    r = 3 * c
    return [
        (S8O + (r + 1) * FW, 0),            # m0: A @ s8 | A_lo @ s8 (stride 0)
        (S8O + r * FW - 1, 2),              # m1: -B/8 @ s(y-1,x-1) | +B/8 @ x+1
        (S8O + (r + 1) * FW - 1, 2),        # m2: -B/4 | +B/4 (center row)
        (S8O + (r + 2) * FW - 1, 2),        # m3: -B/8 | +B/8 (y+1 row)
        (S8O + r * FW, 2 * FW),             # m4: -C/4 @ s(y-1) | +C/4 @ s(y+1)
        (V8O + (r + 1) * FW - 1, 2),        # m5: C/8 @ v(x-1) | C/8 @ v(x+1)
    ]


def _build_program(steps, apply_b2, apply_b1, repeats=1):
    nc = bacc.Bacc("TRN2", target_bir_lowering=False, debug=False,
                   num_devices=N_CORES)

    s_d = nc.dram_tensor("s0", [128, S_FREE], dt.float32, kind="ExternalInput")
    fused = not apply_b1 and not apply_b2
    f_d = nc.dram_tensor("fire", [steps, 128, 2 * COMP if fused else COMP],
                         dt.float8e4, kind="ExternalInput")
    tapw_d = nc.dram_tensor("tapw8", [128, NSLOT * 256], dt.float8e4,
                            kind="ExternalInput")
    w2b_d = nc.dram_tensor("w2b", [128, 2 * 120], dt.bfloat16,
                           kind="ExternalInput")
    b2r_d = nc.dram_tensor("b2r", [128, 1], dt.float32, kind="ExternalInput")
    b1_d = nc.dram_tensor("b1v", [128, 1], dt.float32, kind="ExternalInput")
    out_d = nc.dram_tensor("out", [128, SR * W], dt.float32,
                           kind="ExternalOutput")

    DR = mybir.MatmulPerfMode.DoubleRow
    Relu = mybir.ActivationFunctionType.Relu
    Copy = mybir.ActivationFunctionType.Copy

    import os as _os
    _mb = int(_os.environ.get("NCA_MB", "2"))
    _hb = int(_os.environ.get("NCA_HB", "3"))
    _ub = int(_os.environ.get("NCA_UB", "2"))
    _hpsb = int(_os.environ.get("NCA_HPSB", "3"))
    _dpsb = int(_os.environ.get("NCA_DPSB", "1"))
    _dummy = int(_os.environ.get("NCA_DUMMY", "0"))
    _halo8 = _os.environ.get("NCA_HALO8", "dma")   # act|dve|pool|dma
    _v8e = _os.environ.get("NCA_V8", "gg")          # per-cc engine d|g
    _adde = _os.environ.get("NCA_ADD", "g")         # d|g
    _order = _os.environ.get("NCA_ORDER", "zig")  # fixed|zig
    # relu engine per (k, p) index 2k+p: a=ACT, d=DVE, g=GpSimd
    # (fused relu+fire can only run on DVE/GpSimd)
    _rmap = _os.environ.get("NCA_RELU",
                            "gdddddgd" if not (apply_b1 or apply_b2)
                            else "aaaaaaaa")

    with tile.TileContext(nc) as tc:
        with tc.tile_pool(name="persist", bufs=1) as pp, \
             tc.tile_pool(name="fpool", bufs=_mb) as fpool, \
             tc.tile_pool(name="hsb", bufs=_hb) as hsbp, \
             tc.tile_pool(name="upool", bufs=_ub) as upool, \
             tc.tile_pool(name="hps", bufs=_hpsb, space="PSUM") as hps_pool, \
             tc.tile_pool(name="dps", bufs=_dpsb, space="PSUM") as dps_pool, \
             tc.tile_pool(name="dum", bufs=1, space="PSUM") as dum_pool:

            s_sb = pp.tile([128, S_FREE], dt.float32)
            sv8 = pp.tile([128, SV_TOTAL], dt.float8e4)
            tapw8 = pp.tile([128, NSLOT * 256], dt.float8e4)
            w2b = pp.tile([128, 2 * 120], dt.bfloat16)
            b2r = pp.tile([128, 1], dt.float32)
            b1v = pp.tile([128, 1], dt.float32)

            nc.sync.dma_start(s_sb[:], s_d[:])
            nc.sync.dma_start(tapw8[:], tapw_d[:])
            nc.sync.dma_start(w2b[:], w2b_d[:])
            nc.sync.dma_start(b2r[:], b2r_d[:])
            nc.sync.dma_start(b1v[:], b1_d[:])

            # prologue: zero fp8 scratch (guards/gaps), then full-frame casts
            nc.gpsimd.memset(sv8[:], 0)
            nc.scalar.activation(
                sv8[0:120, S8O - 1:S8O + FRAME + 1],
                s_sb[0:120, FRAME_OFF - 1:FRAME_OFF + FRAME + 1], Copy)
            nc.vector.tensor_tensor(
                sv8[0:120, V8O + FW:V8O + FW + COMP],
                s_sb[0:120, FRAME_OFF + 2 * FW:FRAME_OFF + 2 * FW + COMP],
                s_sb[0:120, FRAME_OFF:FRAME_OFF + COMP],
                mybir.AluOpType.subtract)

            def dr_rhs(base, off, delta):
                v = sv8[base:base + 56, off:off + NCH]
                return AP(v.tensor, v.offset,
                          [list(v.ap[0])] + [[delta, 2]] + [[1, NCH]])

            def relu_one(eng, hsb, hps, f_sb, k, p):
                if not fused:
                    ho = hsb[:].rearrange("p (b x) -> p b x", x=NCH)
                    hi = hps[:].rearrange("p (b x) -> p b x", b=2)[:, :, 0:NCH]
                    if eng == "a":
                        nc.scalar.activation(ho, hi, Relu, bias=b1v[:, 0:1],
                                             scale=1.0 / WS)
                    else:
                        e = nc.vector if eng == "d" else nc.gpsimd
                        e.tensor_scalar(ho, hi, 1.0 / WS, 0.0,
                                        mybir.AluOpType.mult,
                                        mybir.AluOpType.max)
                    return
                # fused: hsb = max(hps, 0) * fire/WS, one stt per chunk so
                # the cc0 half overlaps the cc1 taps (DVE: reads PSUM, which
                # GpSimd cannot touch on hardware)
                e = nc.vector
                fbase = (2 * k + p) * 780
                for cc in range(2):
                    e.scalar_tensor_tensor(
                        hsb[:, NCH * cc:NCH * cc + NCH],
                        hps[:, 512 * cc:512 * cc + NCH],
                        0.0,
                        f_sb[:, fbase + NCH * cc:fbase + NCH * cc + NCH],
                        mybir.AluOpType.max,
                        mybir.AluOpType.mult,
                    )

            def compute_pair(k, f_sb, u_sb):
                hsb_list = []
                for p in range(2):
                    base = 64 * p
                    hps = hps_pool.tile([128, 1024], dt.float32, tag="hps")
                    for cc in range(2):
                        c = 2 * k + cc
                        for i, (off, delta) in enumerate(_slot_table(c)):
                            nc.tensor.matmul(
                                hps[:, 512 * cc:512 * cc + NCH],
                                tapw8[base:base + 56,
                                      256 * i:256 * i + 256].rearrange(
                                          "p (j m) -> p j m", j=2),
                                dr_rhs(base, off, delta),
                                start=(i == 0), stop=(i == NSLOT - 1),
                                perf_mode=DR,
                                tile_position=(base, 0),
                            )
                    hsb = hsbp.tile([128, 2 * NCH], dt.bfloat16,
                                    tag=f"hsb{p}")
                    hsb_list.append(hsb)
                    relu_one(_rmap[2 * k + p], hsb, hps, f_sb, k, p)

                dps = dps_pool.tile([128, 1024], dt.float32, tag="dps")
                for cc in range(2):
                    for p in range(2):
                        nc.tensor.matmul(
                            dps[0:120, 512 * cc:512 * cc + NCH],
                            w2b[:, 120 * p:120 * p + 120],
                            hsb_list[p][:, NCH * cc:NCH * cc + NCH],
                            start=(p == 0), stop=(p == 1),
                        )
                if apply_b2:
                    nc.vector.tensor_scalar_add(
                        dps[0:120].rearrange(
                            "p (b x) -> p b x", b=2)[:, :, 0:NCH],
                        dps[0:120].rearrange(
                            "p (b x) -> p b x", b=2)[:, :, 0:NCH],
                        b2r[0:120, 0:1],
                    )
                if fused:
                    return dps
                # u = fire * delta
                nc.vector.tensor_tensor(
                    u_sb[0:120, 780 * k:780 * k + 780].rearrange(
                        "p (b x) -> p b x", x=NCH),
                    f_sb[0:120, 780 * k:780 * k + 780].rearrange(
                        "p (b x) -> p b x", x=NCH),
                    dps[0:120].rearrange("p (b x) -> p b x", b=2)[:, :, 0:NCH],
                    mybir.AluOpType.mult,
                )
                return dps

            def tail_pair(k, u_sb, dps=None):
                """s8 = q8(s + u) straight off the update (no add
                dependency), split so the first rows (which the next step's
                leading matmuls read) depend only on the cc0 half of dps;
                then the fp32 residual add, then edge halos."""
                r0 = (6 * k + 1) * FW
                a = FRAME_OFF + r0
                if fused:
                    stt = nc.vector.scalar_tensor_tensor
                    Mu, Ad = mybir.AluOpType.mult, mybir.AluOpType.add
                    # rows 1..3 of the pair (dps cc0)
                    stt(sv8[0:120, S8O + r0:S8O + r0 + NCH],
                        dps[0:120, 0:NCH], 1.0,
                        s_sb[0:120, a:a + NCH], Mu, Ad)
                    # row 4 (first 130 of dps cc1)
                    stt(sv8[0:120, S8O + r0 + NCH:S8O + r0 + NCH + FW],
                        dps[0:120, 512:512 + FW], 1.0,
                        s_sb[0:120, a + NCH:a + NCH + FW], Mu, Ad)
                    # rows 5..6 (rest of dps cc1)
                    stt(sv8[0:120, S8O + r0 + NCH + FW:S8O + r0 + 780],
                        dps[0:120, 512 + FW:512 + NCH],
                        1.0,
                        s_sb[0:120, a + NCH + FW:a + 780], Mu, Ad)
                    # stage the masked update to SBUF on ACT (PSUM-legal
                    # single-input copy), then the fp32 add on GpSimd
                    uq = u_sb[0:120, 780 * k:780 * k + 780]
                    nc.scalar.activation(
                        uq.rearrange("p (b x) -> p b x", x=NCH),
                        dps[0:120].rearrange(
                            "p (b x) -> p b x", b=2)[:, :, 0:NCH],
                        mybir.ActivationFunctionType.Copy)
                    nc.gpsimd.tensor_add(
                        s_sb[0:120, a:a + 780],
                        s_sb[0:120, a:a + 780],
                        uq)
                else:
                    nc.gpsimd.scalar_tensor_tensor(
                        sv8[0:120, S8O + r0:S8O + r0 + 780],
                        u_sb[0:120, 780 * k:780 * k + 780],
                        1.0,
                        s_sb[0:120, a:a + 780],
                        mybir.AluOpType.mult,
                        mybir.AluOpType.add,
                    )
                    nc.gpsimd.tensor_add(
                        s_sb[0:120, a:a + 780],
                        s_sb[0:120, a:a + 780],
                        u_sb[0:120, 780 * k:780 * k + 780],
                    )
                def halo8(dst_off, src_off, row8):
                    if _halo8 == "dma":
                        for g in range(3):
                            lo = 32 * g if row8 == 25 * FW else 32 * (g + 1)
                            hi = 32 * (g + 1) if row8 == 25 * FW else 32 * g
                            nc.sync.dma_start(
                                sv8[lo:lo + 24, S8O + row8:S8O + row8 + FW],
                                sv8[hi:hi + 24, S8O + src_off:
                                    S8O + src_off + FW])
                    elif _halo8 == "act":
                        nc.scalar.activation(
                            sv8[0:120, S8O + row8:S8O + row8 + FW],
                            s_sb[0:120, FRAME_OFF + row8:
                                 FRAME_OFF + row8 + FW],
                            mybir.ActivationFunctionType.Copy)
                    else:
                        e = nc.vector if _halo8 == "dve" else nc.gpsimd
                        e.tensor_copy(
                            sv8[0:120, S8O + row8:S8O + row8 + FW],
                            s_sb[0:120, FRAME_OFF + row8:
                                 FRAME_OFF + row8 + FW])

                if k == 0:
                    # fp8 row 1 -> neighbor's halo row 25
                    halo8(25 * FW, FW, 25 * FW)
                if k == NCHUNK // 2 - 1:
                    # fp8 row 24 -> neighbor's halo row 0
                    halo8(0, 24 * FW, 0)

            def v8_pair(k):
                # v8 = s8(r+1) - s8(r-1) from the fp8 mirror (skips the fp32
                # add in the dependency chain), per-cc so m5 unblocks early
                r0 = (6 * k + 1) * FW
                for cc in range(2):
                    o = r0 + NCH * cc
                    e = nc.vector if _v8e[cc] == "d" else nc.gpsimd
                    e.tensor_tensor(
                        sv8[0:120, V8O + o:V8O + o + NCH],
                        sv8[0:120, S8O + o + FW:S8O + o + FW + NCH],
                        sv8[0:120, S8O + o - FW:S8O + o - FW + NCH],
                        mybir.AluOpType.subtract)

            # optional dummy DoubleRow matmuls (NCA_DUMMY>0): filler PE work
            # reading static weights into a scratch psum bank
            if _dummy:
                dum = dum_pool.tile([128, 512], dt.float32)
                dv = tapw8[0:56, 0:1024]
                dum_rhs = AP(dv.tensor, dv.offset,
                             [list(dv.ap[0])] + [[0, 2]] + [[1, 512]])
                dum_lhsT = tapw8[0:56, 0:256].rearrange(
                    "p (j m) -> p j m", j=2)

            for t in range(steps * repeats):
                last = t == steps * repeats - 1
                t = t % steps
                f_sb = fpool.tile([128, 2 * COMP if fused else COMP],
                                  dt.float8e4, tag="f")
                nc.sync.dma_start(f_sb[:], f_d[t])

                u_sb = upool.tile([128, COMP], dt.float32, tag="u")

                dpss = {}
                if _order == "fixed":
                    # fixed pair order [1,0,2,3] with staged tails: regions
                    # the next step's leading pairs read (s8 0..2, v8 1) are
                    # refreshed before this step's end; end-gated pieces
                    # (tail 3, edge halos, v8 0/2/3) resolve under the next
                    # step's leading pairs.
                    dpss[1] = compute_pair(1, f_sb, u_sb)
                    dpss[0] = compute_pair(0, f_sb, u_sb)
                    tail_pair(0, u_sb, dpss[0])
                    dpss[2] = compute_pair(2, f_sb, u_sb)
                    tail_pair(1, u_sb, dpss[1])
                    dpss[3] = compute_pair(3, f_sb, u_sb)
                    tail_pair(2, u_sb, dpss[2])
                    v8_pair(1)
                    tail_pair(3, u_sb, dpss[3])
                    v8_pair(0)
                    v8_pair(2)
                    v8_pair(3)
                else:
                    korder = (list(range(NCHUNK // 2)) if t % 2 == 0
                              else list(range(NCHUNK // 2 - 1, -1, -1)))
                    for i, k in enumerate(korder):
                        dpss[k] = compute_pair(k, f_sb, u_sb)
                        if i >= 1:
                            kp = korder[i - 1]
                            tail_pair(kp, u_sb, dpss[kp])
                        if i >= 2:
                            v8_pair(korder[i - 2])
                    tail_pair(korder[-1], u_sb, dpss[korder[-1]])
                    v8_pair(korder[-2])
                    v8_pair(korder[-1])

                if _dummy and not last:
                    for _ in range(_dummy):
                        nc.tensor.matmul(dum[:, 0:512], dum_lhsT, dum_rhs,
                                         start=True, stop=True, perf_mode=DR,
                                         tile_position=(0, 0))

            # write back real pixels (frame rows 1..24, cols 1..128)
            a0 = FRAME_OFF + FW + 1
            nc.sync.dma_start(
                out_d[:].rearrange("p (r x) -> p r x", x=W),
                s_sb[:, a0:a0 + SR * FW].rearrange(
                    "p (r x) -> p r x", x=FW)[:, :, 0:W],
            )

    nc.compile()
    return nc


def _prep_weights(w1, b1, w2, b2):
    f8 = np.dtype(dt.np(dt.float8e4))
    bf = np.dtype(dt.np(dt.bfloat16))

    def q8(x):
        return np.asarray(x, np.float32).astype(f8).astype(np.float32)

    A = np.ascontiguousarray(w1[:, 0::3]).astype(np.float32)   # [64, 24]
    Bm = np.ascontiguousarray(w1[:, 1::3]).astype(np.float32)
    Cm = np.ascontiguousarray(w1[:, 2::3]).astype(np.float32)

    qA = q8(A * WS)
    qAlo = q8(A * WS - qA)
    qB = q8(Bm * WS)
    qC = q8(Cm * WS)

    # per-slot (j0, j1) weight matrices [64, 24]; values are already the
    # fp8-representable numbers (exact power-of-2 scalings of qA/qB/qC)
    slots = [
        (qA, qAlo),
        (-qB / 8, qB / 8),
        (-qB / 4, qB / 4),
        (-qB / 8, qB / 8),
        (-qC / 4, qC / 4),
        (qC / 8, qC / 8),
    ]

    tapw8 = np.zeros((128, NSLOT * 256), np.float32)
    for i, (w0, w1s) in enumerate(slots):
        for p in range(2):
            base = 64 * p
            for j, wj in enumerate((w0, w1s)):
                blk = 256 * i + 128 * j
                tapw8[base:base + 24, blk:blk + 64] = wj.T
                tapw8[base + 32:base + 56, blk + 64:blk + 128] = wj.T
    tapw8 = tapw8.astype(f8)

    w2b = np.zeros((128, 2 * 120), np.float32)
    for p in range(2):
        ge, go = 2 * p, 2 * p + 1
        w2b[0:64, 120 * p + 32 * ge:120 * p + 32 * ge + 24] = w2.T
        w2b[64:128, 120 * p + 32 * go:120 * p + 32 * go + 24] = w2.T
    w2b = w2b.astype(bf)

    b2r = np.zeros((128, 1), np.float32)
    b1v = np.zeros((128, 1), np.float32)
    for g in range(4):
        b2r[32 * g:32 * g + 24, 0] = b2
    b1v[0:64, 0] = b1
    b1v[64:128, 0] = b1
    return tapw8, w2b, b2r, b1v


def _prep_state(state):
    """state (B, C, H, W) -> per-core [128, S_FREE] framed slabs."""
    bufs = []
    for core in range(N_CORES):
        b = core // 2
        top = (core % 2) == 0
        r0 = 0 if top else H - SH
        buf = np.zeros((128, S_FREE), np.float32)
        for ch in range(C):
            full = np.zeros((SH + 2, FW), np.float32)
            full[1:SH + 1, 1:W + 1] = state[b, ch, r0:r0 + SH, :]
            if r0 > 0:
                full[0, 1:W + 1] = state[b, ch, r0 - 1, :]
            if r0 + SH < H:
                full[SH + 1, 1:W + 1] = state[b, ch, r0 + SH, :]
            for g in range(4):
                fr = full[g * SR:g * SR + FR, :]
                buf[32 * g + ch, FRAME_OFF:FRAME_OFF + FRAME] = fr.reshape(-1)
        bufs.append(buf)
    return bufs


def _prep_masks(masks, fused):
    """masks (S, B, 1, H, W) -> per-core fp8 fire buffers.

    Unfused: [S, 128, COMP] replicated across channel lanes (quadrant
    layout).  Fused: [S, 128, 2*COMP] in the h layout — partition halves
    are the even/odd slab of a pair, free dim is (k, p, cc, pixel), and
    the values carry the 1/WS relu unscale.
    """
    f8 = np.dtype(dt.np(dt.float8e4))
    S = masks.shape[0]
    bufs = []
    for core in range(N_CORES):
        b = core // 2
        top = (core % 2) == 0
        r0 = 0 if top else H - SH
        mrows = np.zeros((S, SH, FW), np.float32)
        mrows[:, :, 1:W + 1] = (masks[:, b, 0, r0:r0 + SH, :]
                                < FIRE_RATE).astype(np.float32)
        if not fused:
            fire = np.zeros((S, 128, COMP), np.float32)
            for g in range(4):
                seg = mrows[:, g * SR:(g + 1) * SR, :].reshape(S, COMP)
                fire[:, 32 * g:32 * g + C, :] = seg[:, None, :]
            bufs.append(fire.astype(f8))
            continue
        fire = np.zeros((S, 128, 2 * COMP), np.float32)
        for k in range(4):
            for p in range(2):
                base = (2 * k + p) * 780
                for half, g in ((0, 2 * p), (64, 2 * p + 1)):
                    seg = mrows[:, g * SR + 6 * k:g * SR + 6 * k + 6, :]
                    fire[:, half:half + 64, base:base + 780] = \
                        seg.reshape(S, 1, 780) / WS
        bufs.append(fire.astype(f8))
    return bufs


def _prep_all(state, w1, b1, w2, b2, masks):
    tapw8, w2b, b2r, b1v = _prep_weights(w1, b1, w2, b2)
    s_bufs = _prep_state(state)
    fused = not (np.any(np.asarray(b1) != 0) or np.any(np.asarray(b2) != 0))
    f_bufs = _prep_masks(masks, fused)
    in_maps = []
    for core in range(N_CORES):
        in_maps.append({
            "s0": s_bufs[core],
            "fire": f_bufs[core],
            "tapw8": tapw8,
            "w2b": w2b,
            "b2r": b2r,
            "b1v": b1v,
        })
    return in_maps


def _prog_key(masks, b1, b2):
    import os as _os
    steps = masks.shape[0]
    apply_b2 = bool(np.any(b2 != 0))
    apply_b1 = bool(np.any(b1 != 0))
    repeats = int(_os.environ.get("NCA_REPEAT", "1"))
    return ("prog", steps, apply_b2, apply_b1, repeats)


def _get_program(masks, b1, b2):
    key = _prog_key(masks, b1, b2)
    if key not in _cache:
        _cache[key] = _build_program(key[1], key[2], key[3], key[4])
    return _cache[key]


def kernel(state, w1, b1, w2, b2, masks):
    state = np.asarray(state)
    w1, b1 = np.asarray(w1), np.asarray(b1)
    w2, b2 = np.asarray(w2), np.asarray(b2)
    masks = np.asarray(masks)
    nc = _get_program(masks, b1, b2)

    from concourse.bass_utils import run_bass_kernel_spmd

    in_maps = _prep_all(state, w1, b1, w2, b2, masks)

    import os
    trace = bool(os.environ.get("NCA_TRACE"))
    kw = {}
    if trace:
        kw["trace"] = True
        if os.environ.get("NCA_TRACE_DIR"):
            kw["tmpdir"] = os.environ["NCA_TRACE_DIR"]
    res = run_bass_kernel_spmd(nc, in_maps, list(range(N_CORES)), **kw)
    global LAST_EXEC_NS
    LAST_EXEC_NS = res.exec_time_ns

    out = np.zeros((B, C, H, W), np.float32)
    for core in range(N_CORES):
        o = res.results[core]["out"]  # [128, SR*W]
        b = core // 2
        top = (core % 2) == 0
        r0 = 0 if top else H - SH
        own0 = 0 if top else H // 2
        for g in range(4):
            rows = o[32 * g:32 * g + 24].reshape(C, SR, W)
            g0 = r0 + g * SR
            lo = max(g0, own0)
            hi = min(g0 + SR, own0 + H // 2)
            if lo < hi:
                out[b, :, lo:hi, :] = rows[:, lo - g0:hi - g0, :]
    return out


# revision 25
# speedup vs baseline: 2.1097x; 1.2635x over previous
"""BasicNCA (neural cellular automaton) Trainium2 kernel, 8-core SPMD, v2.

Reference computation (per step, 32 steps):
  p  = depthwise3x3(s, [identity, sobel_x, sobel_y])   # (B, 3C, H, W)
  h  = relu(w1 @ p + b1)                               # (B, 64, H, W)
  d  = w2 @ h + b2                                     # (B, C, H, W)
  s += d * (mask < 0.5)

v2 strategy (vs the fp32r 9-tap baseline):
- fp8e4m3 DoubleRow matmuls: each PE instruction contracts TWO k-tiles
  (weight slots) at 0.5 cycles/output-row, 4x the fp32r tap rate.  The
  separable sobel structure packs the whole perception+w1 layer into 6
  DoubleRow matmuls per chunk (vs 9 full-rate fp32r taps):
    sobel_x = [1,2,1]^T (x) [-1,0,1]/8 -> six +-B/8-weighted s8 windows
    sobel_y = [-1,0,1]^T (x) [1,2,1]/8 -> v8 = s(y+1)-s(y-1) materialized
      on DVE, windows v8@{x-1,x+1} + center expanded to s8 rows +-1
    identity -> A @ s8, plus a hi-lo correction slot A_lo @ (s/16) that
      recovers most of the fp8 weight-quantization error (the identity
      path dominates it; sobel matrices B, C enter /8 so their error is
      small).  Weight slots hold q8(M*WS) and exact power-of-2 scalings;
      the 1/WS unscale folds into the relu's activation scale.
- h is bf16 (relu output); the 1x1 layer-2 matmul runs in bf16 at full
  rate, K=128 over both slabs' h, M=120 as in the baseline.
- fire masks are precomputed on the host as fp8 0/1, quartering the
  per-step DMA stream vs fp32 uniforms.
- Elementwise work is spread across all three non-PE engines: v8 + fire
  mult (+2 relu chunks) on DVE, s8/s8d casts (+4 relu chunks) on ACT,
  s += u adds (+2 relu chunks) on GpSimd.  With b1 == 0 relu is a plain
  scale+max; a nonzero b1 falls back to all-ACT activation relu.
- Sharding unchanged: core i = batch i//2, H-half i%2, 96-row slab with
  a 32-row taper of redundant compute (no cross-core exchange); 4
  sub-slabs of 24 rows on the SBUF partition quadrants; 3-row chunks.
"""

import sys

sys.path.insert(0, "/opt/trn_rl_repo")

import numpy as np

import concourse.bass as bass
import concourse.bacc as bacc
import concourse.tile as tile
import concourse.mybir as mybir
from concourse.ap import AP

dt = mybir.dt

B, C, H, W = 4, 24, 128, 128
HID = 64
FIRE_RATE = 0.5
N_CORES = 8

SH = 96            # slab rows per core (64 own + 32 taper)
SR = 24            # rows per sub-slab (one partition quadrant)
FW = W + 2         # padded row width (130)
FR = SR + 2        # frame rows per sub-slab (26)
FRAME_OFF = 4      # leading guard elems so tap offset -1 stays in-bounds
FRAME = FR * FW    # 3380
S_FREE = FRAME_OFF + FRAME + 4
COMP = SR * FW     # 3120 compact free size (real rows 0..23)
NCH = 390          # chunk = 3 rows
NCHUNK = COMP // NCH  # 8

# fp8 scratch tile: three FRAME-sized regions (s8, v8, s8d) at a common
# pitch so the hi-lo slot's k-tile stride is constant.
G8 = 8
F8T = FRAME + 2 * G8          # 3396 region pitch
S8O = G8                       # s8 = q8(s)
V8O = G8 + F8T                 # v8 = q8(s(y+1) - s(y-1))
SV_TOTAL = 2 * F8T + 2 * G8

WS = 32.0          # weight pre-scale keeping q8(M*WS) in e4m3 normal range
NSLOT = 6          # DoubleRow matmuls per chunk

LAST_EXEC_NS = None
_cache = {}


def _slot_table(c):
    """Per-chunk DoubleRow matmul table: (j0 offset, k-tile stride).

    Window offsets are relative to the sv8 tile; weights live in tapw8
    blocks of 256 (= 2 k-tiles x 128 out) per slot, see _prep_weights.
    """
    r = 3 * c
    return [
        (S8O + (r + 1) * FW, 0),            # m0: A @ s8 | A_lo @ s8 (stride 0)
        (S8O + r * FW - 1, 2),              # m1: -B/8 @ s(y-1,x-1) | +B/8 @ x+1
        (S8O + (r + 1) * FW - 1, 2),        # m2: -B/4 | +B/4 (center row)
        (S8O + (r + 2) * FW - 1, 2),        # m3: -B/8 | +B/8 (y+1 row)
        (S8O + r * FW, 2 * FW),             # m4: -C/4 @ s(y-1) | +C/4 @ s(y+1)
        (V8O + (r + 1) * FW - 1, 2),        # m5: C/8 @ v(x-1) | C/8 @ v(x+1)
    ]


def _build_program(steps, apply_b2, apply_b1, repeats=1):
    nc = bacc.Bacc("TRN2", target_bir_lowering=False, debug=False,
                   num_devices=N_CORES)

    s_d = nc.dram_tensor("s0", [128, S_FREE], dt.float32, kind="ExternalInput")
    f_d = nc.dram_tensor("fire", [steps, 128, COMP], dt.float8e4,
                         kind="ExternalInput")
    tapw_d = nc.dram_tensor("tapw8", [128, NSLOT * 256], dt.float8e4,
                            kind="ExternalInput")
    w2b_d = nc.dram_tensor("w2b", [128, 2 * 120], dt.bfloat16,
                           kind="ExternalInput")
    b2r_d = nc.dram_tensor("b2r", [128, 1], dt.float32, kind="ExternalInput")
    b1_d = nc.dram_tensor("b1v", [128, 1], dt.float32, kind="ExternalInput")
    out_d = nc.dram_tensor("out", [128, SR * W], dt.float32,
                           kind="ExternalOutput")

    DR = mybir.MatmulPerfMode.DoubleRow
    Relu = mybir.ActivationFunctionType.Relu
    Copy = mybir.ActivationFunctionType.Copy

    import os as _os
    _mb = int(_os.environ.get("NCA_MB", "2"))
    _hb = int(_os.environ.get("NCA_HB", "3"))
    _ub = int(_os.environ.get("NCA_UB", "2"))
    _hpsb = int(_os.environ.get("NCA_HPSB", "3"))
    _dpsb = int(_os.environ.get("NCA_DPSB", "1"))
    _dummy = int(_os.environ.get("NCA_DUMMY", "0"))
    _halo8 = _os.environ.get("NCA_HALO8", "dma")   # act|dve|pool|dma
    _v8e = _os.environ.get("NCA_V8", "dd")          # per-cc engine d|g
    _adde = _os.environ.get("NCA_ADD", "g")         # d|g
    _order = _os.environ.get("NCA_ORDER", "zig")  # fixed|zig
    # relu engine per (k, p) index 2k+p: a=ACT, d=DVE, g=GpSimd
    # (fused relu+fire can only run on DVE/GpSimd)
    _rmap = _os.environ.get("NCA_RELU",
                            "gdddddgd" if not (apply_b1 or apply_b2)
                            else "aaaaaaaa")

    with tile.TileContext(nc) as tc:
        with tc.tile_pool(name="persist", bufs=1) as pp, \
             tc.tile_pool(name="fpool", bufs=_mb) as fpool, \
             tc.tile_pool(name="hsb", bufs=_hb) as hsbp, \
             tc.tile_pool(name="upool", bufs=_ub) as upool, \
             tc.tile_pool(name="hps", bufs=_hpsb, space="PSUM") as hps_pool, \
             tc.tile_pool(name="dps", bufs=_dpsb, space="PSUM") as dps_pool, \
             tc.tile_pool(name="dum", bufs=1, space="PSUM") as dum_pool:

            s_sb = pp.tile([128, S_FREE], dt.float32)
            sv8 = pp.tile([128, SV_TOTAL], dt.float8e4)
            tapw8 = pp.tile([128, NSLOT * 256], dt.float8e4)
            w2b = pp.tile([128, 2 * 120], dt.bfloat16)
            b2r = pp.tile([128, 1], dt.float32)
            b1v = pp.tile([128, 1], dt.float32)

            nc.sync.dma_start(s_sb[:], s_d[:])
            nc.sync.dma_start(tapw8[:], tapw_d[:])
            nc.sync.dma_start(w2b[:], w2b_d[:])
            nc.sync.dma_start(b2r[:], b2r_d[:])
            nc.sync.dma_start(b1v[:], b1_d[:])

            # prologue: zero fp8 scratch (guards/gaps), then full-frame casts
            nc.gpsimd.memset(sv8[:], 0)
            nc.scalar.activation(
                sv8[0:120, S8O - 1:S8O + FRAME + 1],
                s_sb[0:120, FRAME_OFF - 1:FRAME_OFF + FRAME + 1], Copy)
            nc.vector.tensor_tensor(
                sv8[0:120, V8O + FW:V8O + FW + COMP],
                s_sb[0:120, FRAME_OFF + 2 * FW:FRAME_OFF + 2 * FW + COMP],
                s_sb[0:120, FRAME_OFF:FRAME_OFF + COMP],
                mybir.AluOpType.subtract)

            def dr_rhs(base, off, delta):
                v = sv8[base:base + 56, off:off + NCH]
                return AP(v.tensor, v.offset,
                          [list(v.ap[0])] + [[delta, 2]] + [[1, NCH]])

            def relu_one(eng, hsb, hps, f_sb, k, p):
                # ACT relu per chunk half: the cc0 half overlaps cc1 taps
                for cc in range(2):
                    nc.scalar.activation(
                        hsb[:, NCH * cc:NCH * cc + NCH],
                        hps[:, 512 * cc:512 * cc + NCH],
                        Relu, bias=b1v[:, 0:1], scale=1.0 / WS)

            def compute_pair(k, f_sb, u_sb):
                hsb_list = []
                for p in range(2):
                    base = 64 * p
                    hps = hps_pool.tile([128, 1024], dt.float32, tag="hps")
                    for cc in range(2):
                        c = 2 * k + cc
                        for i, (off, delta) in enumerate(_slot_table(c)):
                            nc.tensor.matmul(
                                hps[:, 512 * cc:512 * cc + NCH],
                                tapw8[base:base + 56,
                                      256 * i:256 * i + 256].rearrange(
                                          "p (j m) -> p j m", j=2),
                                dr_rhs(base, off, delta),
                                start=(i == 0), stop=(i == NSLOT - 1),
                                perf_mode=DR,
                                tile_position=(base, 0),
                            )
                    hsb = hsbp.tile([128, 2 * NCH], dt.bfloat16,
                                    tag=f"hsb{p}")
                    hsb_list.append(hsb)
                    relu_one(_rmap[2 * k + p], hsb, hps, f_sb, k, p)

                dps = dps_pool.tile([128, 1024], dt.float32, tag="dps")
                for cc in range(2):
                    for p in range(2):
                        nc.tensor.matmul(
                            dps[0:120, 512 * cc:512 * cc + NCH],
                            w2b[:, 120 * p:120 * p + 120],
                            hsb_list[p][:, NCH * cc:NCH * cc + NCH],
                            start=(p == 0), stop=(p == 1),
                        )
                if apply_b2:
                    nc.vector.tensor_scalar_add(
                        dps[0:120].rearrange(
                            "p (b x) -> p b x", b=2)[:, :, 0:NCH],
                        dps[0:120].rearrange(
                            "p (b x) -> p b x", b=2)[:, :, 0:NCH],
                        b2r[0:120, 0:1],
                    )
                # u = fire * delta, per chunk half (DVE: reads PSUM)
                for cc in range(2):
                    nc.vector.tensor_tensor(
                        u_sb[0:120, 780 * k + NCH * cc:
                             780 * k + NCH * cc + NCH],
                        f_sb[0:120, 780 * k + NCH * cc:
                             780 * k + NCH * cc + NCH],
                        dps[0:120, 512 * cc:512 * cc + NCH],
                        mybir.AluOpType.mult,
                    )
                return dps
                # u = fire * delta
                nc.vector.tensor_tensor(
                    u_sb[0:120, 780 * k:780 * k + 780].rearrange(
                        "p (b x) -> p b x", x=NCH),
                    f_sb[0:120, 780 * k:780 * k + 780].rearrange(
                        "p (b x) -> p b x", x=NCH),
                    dps[0:120].rearrange("p (b x) -> p b x", b=2)[:, :, 0:NCH],
                    mybir.AluOpType.mult,
                )
                return dps

            def tail_pair(k, u_sb, dps=None):
                """s8 = q8(s + u) from the SBUF update on GpSimd, split so
                the rows the next step's leading matmuls need depend only on
                the cc0 fire half; then the fp32 residual add."""
                r0 = (6 * k + 1) * FW
                a = FRAME_OFF + r0
                ub = 780 * k
                # rows 1..3 (u cc0)
                nc.gpsimd.tensor_add(
                    sv8[0:120, S8O + r0:S8O + r0 + NCH],
                    u_sb[0:120, ub:ub + NCH],
                    s_sb[0:120, a:a + NCH])
                # rows 4..6 (u cc1)
                nc.gpsimd.tensor_add(
                    sv8[0:120, S8O + r0 + NCH:S8O + r0 + 780],
                    u_sb[0:120, ub + NCH:ub + 780],
                    s_sb[0:120, a + NCH:a + 780])
                nc.gpsimd.tensor_add(
                    s_sb[0:120, a:a + 780],
                    s_sb[0:120, a:a + 780],
                    u_sb[0:120, ub:ub + 780],
                )
                def halo8(dst_off, src_off, row8):
                    if _halo8 == "dma":
                        for g in range(3):
                            lo = 32 * g if row8 == 25 * FW else 32 * (g + 1)
                            hi = 32 * (g + 1) if row8 == 25 * FW else 32 * g
                            nc.sync.dma_start(
                                sv8[lo:lo + 24, S8O + row8:S8O + row8 + FW],
                                sv8[hi:hi + 24, S8O + src_off:
                                    S8O + src_off + FW])
                    elif _halo8 == "act":
                        nc.scalar.activation(
                            sv8[0:120, S8O + row8:S8O + row8 + FW],
                            s_sb[0:120, FRAME_OFF + row8:
                                 FRAME_OFF + row8 + FW],
                            mybir.ActivationFunctionType.Copy)
                    else:
                        e = nc.vector if _halo8 == "dve" else nc.gpsimd
                        e.tensor_copy(
                            sv8[0:120, S8O + row8:S8O + row8 + FW],
                            s_sb[0:120, FRAME_OFF + row8:
                                 FRAME_OFF + row8 + FW])

                if k == 0:
                    # fp8 row 1 -> neighbor's halo row 25
                    halo8(25 * FW, FW, 25 * FW)
                if k == NCHUNK // 2 - 1:
                    # fp8 row 24 -> neighbor's halo row 0
                    halo8(0, 24 * FW, 0)

            def v8_pair(k):
                # v8 = s8(r+1) - s8(r-1) from the fp8 mirror (skips the fp32
                # add in the dependency chain), per-cc so m5 unblocks early
                r0 = (6 * k + 1) * FW
                for cc in range(2):
                    o = r0 + NCH * cc
                    e = nc.vector if _v8e[cc] == "d" else nc.gpsimd
                    e.tensor_tensor(
                        sv8[0:120, V8O + o:V8O + o + NCH],
                        sv8[0:120, S8O + o + FW:S8O + o + FW + NCH],
                        sv8[0:120, S8O + o - FW:S8O + o - FW + NCH],
                        mybir.AluOpType.subtract)

            # optional dummy DoubleRow matmuls (NCA_DUMMY>0): filler PE work
            # reading static weights into a scratch psum bank
            if _dummy:
                dum = dum_pool.tile([128, 512], dt.float32)
                dv = tapw8[0:56, 0:1024]
                dum_rhs = AP(dv.tensor, dv.offset,
                             [list(dv.ap[0])] + [[0, 2]] + [[1, 512]])
                dum_lhsT = tapw8[0:56, 0:256].rearrange(
                    "p (j m) -> p j m", j=2)

            for t in range(steps * repeats):
                last = t == steps * repeats - 1
                t = t % steps
                f_sb = fpool.tile([128, COMP], dt.float8e4, tag="f")
                nc.sync.dma_start(f_sb[:], f_d[t])

                u_sb = upool.tile([128, COMP], dt.float32, tag="u")

                dpss = {}
                if _order == "fixed":
                    # fixed pair order [1,0,2,3] with staged tails: regions
                    # the next step's leading pairs read (s8 0..2, v8 1) are
                    # refreshed before this step's end; end-gated pieces
                    # (tail 3, edge halos, v8 0/2/3) resolve under the next
                    # step's leading pairs.
                    dpss[1] = compute_pair(1, f_sb, u_sb)
                    dpss[0] = compute_pair(0, f_sb, u_sb)
                    tail_pair(0, u_sb, dpss[0])
                    dpss[2] = compute_pair(2, f_sb, u_sb)
                    tail_pair(1, u_sb, dpss[1])
                    dpss[3] = compute_pair(3, f_sb, u_sb)
                    tail_pair(2, u_sb, dpss[2])
                    v8_pair(1)
                    tail_pair(3, u_sb, dpss[3])
                    v8_pair(0)
                    v8_pair(2)
                    v8_pair(3)
                else:
                    korder = (list(range(NCHUNK // 2)) if t % 2 == 0
                              else list(range(NCHUNK // 2 - 1, -1, -1)))
                    for i, k in enumerate(korder):
                        dpss[k] = compute_pair(k, f_sb, u_sb)
                        if i >= 1:
                            kp = korder[i - 1]
                            tail_pair(kp, u_sb, dpss[kp])
                        if i >= 2:
                            v8_pair(korder[i - 2])
                    tail_pair(korder[-1], u_sb, dpss[korder[-1]])
                    v8_pair(korder[-2])
                    v8_pair(korder[-1])

                if _dummy and not last:
                    for _ in range(_dummy):
                        nc.tensor.matmul(dum[:, 0:512], dum_lhsT, dum_rhs,
                                         start=True, stop=True, perf_mode=DR,
                                         tile_position=(0, 0))

            # write back real pixels (frame rows 1..24, cols 1..128)
            a0 = FRAME_OFF + FW + 1
            nc.sync.dma_start(
                out_d[:].rearrange("p (r x) -> p r x", x=W),
                s_sb[:, a0:a0 + SR * FW].rearrange(
                    "p (r x) -> p r x", x=FW)[:, :, 0:W],
            )

    nc.compile()
    return nc


def _prep_weights(w1, b1, w2, b2):
    f8 = np.dtype(dt.np(dt.float8e4))
    bf = np.dtype(dt.np(dt.bfloat16))

    def q8(x):
        return np.asarray(x, np.float32).astype(f8).astype(np.float32)

    A = np.ascontiguousarray(w1[:, 0::3]).astype(np.float32)   # [64, 24]
    Bm = np.ascontiguousarray(w1[:, 1::3]).astype(np.float32)
    Cm = np.ascontiguousarray(w1[:, 2::3]).astype(np.float32)

    qA = q8(A * WS)
    qAlo = q8(A * WS - qA)
    qB = q8(Bm * WS)
    qC = q8(Cm * WS)

    # per-slot (j0, j1) weight matrices [64, 24]; values are already the
    # fp8-representable numbers (exact power-of-2 scalings of qA/qB/qC)
    slots = [
        (qA, qAlo),
        (-qB / 8, qB / 8),
        (-qB / 4, qB / 4),
        (-qB / 8, qB / 8),
        (-qC / 4, qC / 4),
        (qC / 8, qC / 8),
    ]

    tapw8 = np.zeros((128, NSLOT * 256), np.float32)
    for i, (w0, w1s) in enumerate(slots):
        for p in range(2):
            base = 64 * p
            for j, wj in enumerate((w0, w1s)):
                blk = 256 * i + 128 * j
                tapw8[base:base + 24, blk:blk + 64] = wj.T
                tapw8[base + 32:base + 56, blk + 64:blk + 128] = wj.T
    tapw8 = tapw8.astype(f8)

    w2b = np.zeros((128, 2 * 120), np.float32)
    for p in range(2):
        ge, go = 2 * p, 2 * p + 1
        w2b[0:64, 120 * p + 32 * ge:120 * p + 32 * ge + 24] = w2.T
        w2b[64:128, 120 * p + 32 * go:120 * p + 32 * go + 24] = w2.T
    w2b = w2b.astype(bf)

    b2r = np.zeros((128, 1), np.float32)
    b1v = np.zeros((128, 1), np.float32)
    for g in range(4):
        b2r[32 * g:32 * g + 24, 0] = b2
    b1v[0:64, 0] = b1
    b1v[64:128, 0] = b1
    return tapw8, w2b, b2r, b1v


def _prep_state(state):
    """state (B, C, H, W) -> per-core [128, S_FREE] framed slabs."""
    bufs = []
    for core in range(N_CORES):
        b = core // 2
        top = (core % 2) == 0
        r0 = 0 if top else H - SH
        buf = np.zeros((128, S_FREE), np.float32)
        for ch in range(C):
            full = np.zeros((SH + 2, FW), np.float32)
            full[1:SH + 1, 1:W + 1] = state[b, ch, r0:r0 + SH, :]
            if r0 > 0:
                full[0, 1:W + 1] = state[b, ch, r0 - 1, :]
            if r0 + SH < H:
                full[SH + 1, 1:W + 1] = state[b, ch, r0 + SH, :]
            for g in range(4):
                fr = full[g * SR:g * SR + FR, :]
                buf[32 * g + ch, FRAME_OFF:FRAME_OFF + FRAME] = fr.reshape(-1)
        bufs.append(buf)
    return bufs


def _prep_masks(masks):
    """masks (S, B, 1, H, W) -> per-core [S, 128, COMP] fp8 0/1 fire."""
    f8 = np.dtype(dt.np(dt.float8e4))
    S = masks.shape[0]
    bufs = []
    for core in range(N_CORES):
        b = core // 2
        top = (core % 2) == 0
        r0 = 0 if top else H - SH
        mrows = np.zeros((S, SH, FW), np.float32)
        mrows[:, :, 1:W + 1] = (masks[:, b, 0, r0:r0 + SH, :]
                                < FIRE_RATE).astype(np.float32)
        fire = np.zeros((S, 128, COMP), np.float32)
        for g in range(4):
            seg = mrows[:, g * SR:(g + 1) * SR, :].reshape(S, COMP)
            fire[:, 32 * g:32 * g + C, :] = seg[:, None, :]
        bufs.append(fire.astype(f8))
    return bufs


def _prep_all(state, w1, b1, w2, b2, masks):
    tapw8, w2b, b2r, b1v = _prep_weights(w1, b1, w2, b2)
    s_bufs = _prep_state(state)
    f_bufs = _prep_masks(masks)
    in_maps = []
    for core in range(N_CORES):
        in_maps.append({
            "s0": s_bufs[core],
            "fire": f_bufs[core],
            "tapw8": tapw8,
            "w2b": w2b,
            "b2r": b2r,
            "b1v": b1v,
        })
    return in_maps


def _prog_key(masks, b1, b2):
    import os as _os
    steps = masks.shape[0]
    apply_b2 = bool(np.any(b2 != 0))
    apply_b1 = bool(np.any(b1 != 0))
    repeats = int(_os.environ.get("NCA_REPEAT", "1"))
    return ("prog", steps, apply_b2, apply_b1, repeats)


def _get_program(masks, b1, b2):
    key = _prog_key(masks, b1, b2)
    if key not in _cache:
        _cache[key] = _build_program(key[1], key[2], key[3], key[4])
    return _cache[key]


def kernel(state, w1, b1, w2, b2, masks):
    state = np.asarray(state)
    w1, b1 = np.asarray(w1), np.asarray(b1)
    w2, b2 = np.asarray(w2), np.asarray(b2)
    masks = np.asarray(masks)
    nc = _get_program(masks, b1, b2)

    from concourse.bass_utils import run_bass_kernel_spmd

    in_maps = _prep_all(state, w1, b1, w2, b2, masks)

    import os
    trace = bool(os.environ.get("NCA_TRACE"))
    kw = {}
    if trace:
        kw["trace"] = True
        if os.environ.get("NCA_TRACE_DIR"):
            kw["tmpdir"] = os.environ["NCA_TRACE_DIR"]
    res = run_bass_kernel_spmd(nc, in_maps, list(range(N_CORES)), **kw)
    global LAST_EXEC_NS
    LAST_EXEC_NS = res.exec_time_ns

    out = np.zeros((B, C, H, W), np.float32)
    for core in range(N_CORES):
        o = res.results[core]["out"]  # [128, SR*W]
        b = core // 2
        top = (core % 2) == 0
        r0 = 0 if top else H - SH
        own0 = 0 if top else H // 2
        for g in range(4):
            rows = o[32 * g:32 * g + 24].reshape(C, SR, W)
            g0 = r0 + g * SR
            lo = max(g0, own0)
            hi = min(g0 + SR, own0 + H // 2)
            if lo < hi:
                out[b, :, lo:hi, :] = rows[:, lo - g0:hi - g0, :]
    return out


# revision 26
# speedup vs baseline: 2.1254x; 1.0075x over previous
"""BasicNCA (neural cellular automaton) Trainium2 kernel, 8-core SPMD, v2.

Reference computation (per step, 32 steps):
  p  = depthwise3x3(s, [identity, sobel_x, sobel_y])   # (B, 3C, H, W)
  h  = relu(w1 @ p + b1)                               # (B, 64, H, W)
  d  = w2 @ h + b2                                     # (B, C, H, W)
  s += d * (mask < 0.5)

v2 strategy (vs the fp32r 9-tap baseline):
- fp8e4m3 DoubleRow matmuls: each PE instruction contracts TWO k-tiles
  (weight slots) at 0.5 cycles/output-row, 4x the fp32r tap rate.  The
  separable sobel structure packs the whole perception+w1 layer into 6
  DoubleRow matmuls per chunk (vs 9 full-rate fp32r taps):
    sobel_x = [1,2,1]^T (x) [-1,0,1]/8 -> six +-B/8-weighted s8 windows
    sobel_y = [-1,0,1]^T (x) [1,2,1]/8 -> v8 = s(y+1)-s(y-1) materialized
      on DVE, windows v8@{x-1,x+1} + center expanded to s8 rows +-1
    identity -> A @ s8, plus a hi-lo correction slot A_lo @ (s/16) that
      recovers most of the fp8 weight-quantization error (the identity
      path dominates it; sobel matrices B, C enter /8 so their error is
      small).  Weight slots hold q8(M*WS) and exact power-of-2 scalings;
      the 1/WS unscale folds into the relu's activation scale.
- h is bf16 (relu output); the 1x1 layer-2 matmul runs in bf16 at full
  rate, K=128 over both slabs' h, M=120 as in the baseline.
- fire masks are precomputed on the host as fp8 0/1, quartering the
  per-step DMA stream vs fp32 uniforms.
- Elementwise work is spread across all three non-PE engines: v8 + fire
  mult (+2 relu chunks) on DVE, s8/s8d casts (+4 relu chunks) on ACT,
  s += u adds (+2 relu chunks) on GpSimd.  With b1 == 0 relu is a plain
  scale+max; a nonzero b1 falls back to all-ACT activation relu.
- Sharding unchanged: core i = batch i//2, H-half i%2, 96-row slab with
  a 32-row taper of redundant compute (no cross-core exchange); 4
  sub-slabs of 24 rows on the SBUF partition quadrants; 3-row chunks.
"""

import sys

sys.path.insert(0, "/opt/trn_rl_repo")

import numpy as np

import concourse.bass as bass
import concourse.bacc as bacc
import concourse.tile as tile
import concourse.mybir as mybir
from concourse.ap import AP

dt = mybir.dt

B, C, H, W = 4, 24, 128, 128
HID = 64
FIRE_RATE = 0.5
N_CORES = 8

SH = 96            # slab rows per core (64 own + 32 taper)
SR = 24            # rows per sub-slab (one partition quadrant)
FW = W + 2         # padded row width (130)
FR = SR + 2        # frame rows per sub-slab (26)
FRAME_OFF = 4      # leading guard elems so tap offset -1 stays in-bounds
FRAME = FR * FW    # 3380
S_FREE = FRAME_OFF + FRAME + 4
COMP = SR * FW     # 3120 compact free size (real rows 0..23)
NCH = 390          # chunk = 3 rows
NCHUNK = COMP // NCH  # 8

# fp8 scratch tile: three FRAME-sized regions (s8, v8, s8d) at a common
# pitch so the hi-lo slot's k-tile stride is constant.
G8 = 8
F8T = FRAME + 2 * G8          # 3396 region pitch
S8O = G8                       # s8 = q8(s)
V8O = G8 + F8T                 # v8 = q8(s(y+1) - s(y-1))
SV_TOTAL = 2 * F8T + 2 * G8

WS = 32.0          # weight pre-scale keeping q8(M*WS) in e4m3 normal range
NSLOT = 6          # DoubleRow matmuls per chunk

LAST_EXEC_NS = None
_cache = {}


def _slot_table(c):
    """Per-chunk DoubleRow matmul table: (j0 offset, k-tile stride).

    Window offsets are relative to the sv8 tile; weights live in tapw8
    blocks of 256 (= 2 k-tiles x 128 out) per slot, see _prep_weights.
    """
    r = 3 * c
    return [
        (S8O + (r + 1) * FW, 0),            # m0: A @ s8 | A_lo @ s8 (stride 0)
        (S8O + r * FW - 1, 2),              # m1: -B/8 @ s(y-1,x-1) | +B/8 @ x+1
        (S8O + (r + 1) * FW - 1, 2),        # m2: -B/4 | +B/4 (center row)
        (S8O + (r + 2) * FW - 1, 2),        # m3: -B/8 | +B/8 (y+1 row)
        (S8O + r * FW, 2 * FW),             # m4: -C/4 @ s(y-1) | +C/4 @ s(y+1)
        (V8O + (r + 1) * FW - 1, 2),        # m5: C/8 @ v(x-1) | C/8 @ v(x+1)
    ]


def _build_program(steps, apply_b2, apply_b1, repeats=1):
    nc = bacc.Bacc("TRN2", target_bir_lowering=False, debug=False,
                   num_devices=N_CORES)

    s_d = nc.dram_tensor("s0", [128, S_FREE], dt.float32, kind="ExternalInput")
    f_d = nc.dram_tensor("fire", [steps, 128, COMP], dt.float8e4,
                         kind="ExternalInput")
    tapw_d = nc.dram_tensor("tapw8", [128, NSLOT * 256], dt.float8e4,
                            kind="ExternalInput")
    w2b_d = nc.dram_tensor("w2b", [128, 2 * 120], dt.bfloat16,
                           kind="ExternalInput")
    b2r_d = nc.dram_tensor("b2r", [128, 1], dt.float32, kind="ExternalInput")
    b1_d = nc.dram_tensor("b1v", [128, 1], dt.float32, kind="ExternalInput")
    out_d = nc.dram_tensor("out", [128, SR * W], dt.float32,
                           kind="ExternalOutput")

    DR = mybir.MatmulPerfMode.DoubleRow
    Relu = mybir.ActivationFunctionType.Relu
    Copy = mybir.ActivationFunctionType.Copy

    import os as _os
    _mb = int(_os.environ.get("NCA_MB", "2"))
    _hb = int(_os.environ.get("NCA_HB", "3"))
    _ub = int(_os.environ.get("NCA_UB", "2"))
    _hpsb = int(_os.environ.get("NCA_HPSB", "3"))
    _dpsb = int(_os.environ.get("NCA_DPSB", "1"))
    _dummy = int(_os.environ.get("NCA_DUMMY", "0"))
    _halo8 = _os.environ.get("NCA_HALO8", "dma")   # act|dve|pool|dma
    _v8e = _os.environ.get("NCA_V8", "gd")          # per-cc engine d|g
    _adde = _os.environ.get("NCA_ADD", "g")         # d|g
    _order = _os.environ.get("NCA_ORDER", "zig")  # fixed|zig
    # relu engine per (k, p) index 2k+p: a=ACT, d=DVE, g=GpSimd
    # (fused relu+fire can only run on DVE/GpSimd)
    _rmap = _os.environ.get("NCA_RELU",
                            "gdddddgd" if not (apply_b1 or apply_b2)
                            else "aaaaaaaa")

    with tile.TileContext(nc) as tc:
        with tc.tile_pool(name="persist", bufs=1) as pp, \
             tc.tile_pool(name="fpool", bufs=_mb) as fpool, \
             tc.tile_pool(name="hsb", bufs=_hb) as hsbp, \
             tc.tile_pool(name="upool", bufs=_ub) as upool, \
             tc.tile_pool(name="hps", bufs=_hpsb, space="PSUM") as hps_pool, \
             tc.tile_pool(name="dps", bufs=_dpsb, space="PSUM") as dps_pool, \
             tc.tile_pool(name="dum", bufs=1, space="PSUM") as dum_pool:

            s_sb = pp.tile([128, S_FREE], dt.float32)
            sv8 = pp.tile([128, SV_TOTAL], dt.float8e4)
            tapw8 = pp.tile([128, NSLOT * 256], dt.float8e4)
            w2b = pp.tile([128, 2 * 120], dt.bfloat16)
            b2r = pp.tile([128, 1], dt.float32)
            b1v = pp.tile([128, 1], dt.float32)

            nc.sync.dma_start(s_sb[:], s_d[:])
            nc.sync.dma_start(tapw8[:], tapw_d[:])
            nc.sync.dma_start(w2b[:], w2b_d[:])
            nc.sync.dma_start(b2r[:], b2r_d[:])
            nc.sync.dma_start(b1v[:], b1_d[:])

            # prologue: zero fp8 scratch (guards/gaps), then full-frame casts
            nc.gpsimd.memset(sv8[:], 0)
            nc.scalar.activation(
                sv8[0:120, S8O - 1:S8O + FRAME + 1],
                s_sb[0:120, FRAME_OFF - 1:FRAME_OFF + FRAME + 1], Copy)
            nc.vector.tensor_tensor(
                sv8[0:120, V8O + FW:V8O + FW + COMP],
                s_sb[0:120, FRAME_OFF + 2 * FW:FRAME_OFF + 2 * FW + COMP],
                s_sb[0:120, FRAME_OFF:FRAME_OFF + COMP],
                mybir.AluOpType.subtract)

            def dr_rhs(base, off, delta):
                v = sv8[base:base + 56, off:off + NCH]
                return AP(v.tensor, v.offset,
                          [list(v.ap[0])] + [[delta, 2]] + [[1, NCH]])

            def relu_one(eng, hsb, hps, f_sb, k, p):
                # ACT relu per chunk half: the cc0 half overlaps cc1 taps
                for cc in range(2):
                    nc.scalar.activation(
                        hsb[:, NCH * cc:NCH * cc + NCH],
                        hps[:, 512 * cc:512 * cc + NCH],
                        Relu, bias=b1v[:, 0:1], scale=1.0 / WS)

            def compute_pair(k, f_sb, u_sb):
                hsb_list = []
                for p in range(2):
                    base = 64 * p
                    hps = hps_pool.tile([128, 1024], dt.float32, tag="hps")
                    for cc in range(2):
                        c = 2 * k + cc
                        for i, (off, delta) in enumerate(_slot_table(c)):
                            nc.tensor.matmul(
                                hps[:, 512 * cc:512 * cc + NCH],
                                tapw8[base:base + 56,
                                      256 * i:256 * i + 256].rearrange(
                                          "p (j m) -> p j m", j=2),
                                dr_rhs(base, off, delta),
                                start=(i == 0), stop=(i == NSLOT - 1),
                                perf_mode=DR,
                                tile_position=(base, 0),
                            )
                    hsb = hsbp.tile([128, 2 * NCH], dt.bfloat16,
                                    tag=f"hsb{p}")
                    hsb_list.append(hsb)
                    relu_one(_rmap[2 * k + p], hsb, hps, f_sb, k, p)

                dps = dps_pool.tile([128, 1024], dt.float32, tag="dps")
                for cc in range(2):
                    for p in range(2):
                        nc.tensor.matmul(
                            dps[0:120, 512 * cc:512 * cc + NCH],
                            w2b[:, 120 * p:120 * p + 120],
                            hsb_list[p][:, NCH * cc:NCH * cc + NCH],
                            start=(p == 0), stop=(p == 1),
                        )
                if apply_b2:
                    nc.vector.tensor_scalar_add(
                        dps[0:120].rearrange(
                            "p (b x) -> p b x", b=2)[:, :, 0:NCH],
                        dps[0:120].rearrange(
                            "p (b x) -> p b x", b=2)[:, :, 0:NCH],
                        b2r[0:120, 0:1],
                    )
                # u = fire * delta, per chunk half (DVE: reads PSUM)
                for cc in range(2):
                    nc.vector.tensor_tensor(
                        u_sb[0:120, 780 * k + NCH * cc:
                             780 * k + NCH * cc + NCH],
                        f_sb[0:120, 780 * k + NCH * cc:
                             780 * k + NCH * cc + NCH],
                        dps[0:120, 512 * cc:512 * cc + NCH],
                        mybir.AluOpType.mult,
                    )
                return dps
                # u = fire * delta
                nc.vector.tensor_tensor(
                    u_sb[0:120, 780 * k:780 * k + 780].rearrange(
                        "p (b x) -> p b x", x=NCH),
                    f_sb[0:120, 780 * k:780 * k + 780].rearrange(
                        "p (b x) -> p b x", x=NCH),
                    dps[0:120].rearrange("p (b x) -> p b x", b=2)[:, :, 0:NCH],
                    mybir.AluOpType.mult,
                )
                return dps

            def tail_pair(k, u_sb, dps=None):
                """s8 = q8(s + u) from the SBUF update on GpSimd, split so
                the rows the next step's leading matmuls need depend only on
                the cc0 fire half; then the fp32 residual add."""
                r0 = (6 * k + 1) * FW
                a = FRAME_OFF + r0
                ub = 780 * k
                # rows 1..3 (u cc0)
                nc.gpsimd.tensor_add(
                    sv8[0:120, S8O + r0:S8O + r0 + NCH],
                    u_sb[0:120, ub:ub + NCH],
                    s_sb[0:120, a:a + NCH])
                # rows 4..6 (u cc1)
                nc.gpsimd.tensor_add(
                    sv8[0:120, S8O + r0 + NCH:S8O + r0 + 780],
                    u_sb[0:120, ub + NCH:ub + 780],
                    s_sb[0:120, a + NCH:a + 780])
                nc.gpsimd.tensor_add(
                    s_sb[0:120, a:a + 780],
                    s_sb[0:120, a:a + 780],
                    u_sb[0:120, ub:ub + 780],
                )
                def halo8(dst_off, src_off, row8):
                    if _halo8 == "dma":
                        for g in range(3):
                            lo = 32 * g if row8 == 25 * FW else 32 * (g + 1)
                            hi = 32 * (g + 1) if row8 == 25 * FW else 32 * g
                            nc.sync.dma_start(
                                sv8[lo:lo + 24, S8O + row8:S8O + row8 + FW],
                                sv8[hi:hi + 24, S8O + src_off:
                                    S8O + src_off + FW])
                    elif _halo8 == "act":
                        nc.scalar.activation(
                            sv8[0:120, S8O + row8:S8O + row8 + FW],
                            s_sb[0:120, FRAME_OFF + row8:
                                 FRAME_OFF + row8 + FW],
                            mybir.ActivationFunctionType.Copy)
                    else:
                        e = nc.vector if _halo8 == "dve" else nc.gpsimd
                        e.tensor_copy(
                            sv8[0:120, S8O + row8:S8O + row8 + FW],
                            s_sb[0:120, FRAME_OFF + row8:
                                 FRAME_OFF + row8 + FW])

                if k == 0:
                    # fp8 row 1 -> neighbor's halo row 25
                    halo8(25 * FW, FW, 25 * FW)
                if k == NCHUNK // 2 - 1:
                    # fp8 row 24 -> neighbor's halo row 0
                    halo8(0, 24 * FW, 0)

            def v8_pair(k):
                # v8 = s8(r+1) - s8(r-1) from the fp8 mirror (skips the fp32
                # add in the dependency chain), per-cc so m5 unblocks early
                r0 = (6 * k + 1) * FW
                for cc in range(2):
                    o = r0 + NCH * cc
                    e = nc.vector if _v8e[cc] == "d" else nc.gpsimd
                    e.tensor_tensor(
                        sv8[0:120, V8O + o:V8O + o + NCH],
                        sv8[0:120, S8O + o + FW:S8O + o + FW + NCH],
                        sv8[0:120, S8O + o - FW:S8O + o - FW + NCH],
                        mybir.AluOpType.subtract)

            # optional dummy DoubleRow matmuls (NCA_DUMMY>0): filler PE work
            # reading static weights into a scratch psum bank
            if _dummy:
                dum = dum_pool.tile([128, 512], dt.float32)
                dv = tapw8[0:56, 0:1024]
                dum_rhs = AP(dv.tensor, dv.offset,
                             [list(dv.ap[0])] + [[0, 2]] + [[1, 512]])
                dum_lhsT = tapw8[0:56, 0:256].rearrange(
                    "p (j m) -> p j m", j=2)

            for t in range(steps * repeats):
                last = t == steps * repeats - 1
                t = t % steps
                f_sb = fpool.tile([128, COMP], dt.float8e4, tag="f")
                nc.sync.dma_start(f_sb[:], f_d[t])

                u_sb = upool.tile([128, COMP], dt.float32, tag="u")

                dpss = {}
                if _order == "fixed":
                    # fixed pair order [1,0,2,3] with staged tails: regions
                    # the next step's leading pairs read (s8 0..2, v8 1) are
                    # refreshed before this step's end; end-gated pieces
                    # (tail 3, edge halos, v8 0/2/3) resolve under the next
                    # step's leading pairs.
                    dpss[1] = compute_pair(1, f_sb, u_sb)
                    dpss[0] = compute_pair(0, f_sb, u_sb)
                    tail_pair(0, u_sb, dpss[0])
                    dpss[2] = compute_pair(2, f_sb, u_sb)
                    tail_pair(1, u_sb, dpss[1])
                    dpss[3] = compute_pair(3, f_sb, u_sb)
                    tail_pair(2, u_sb, dpss[2])
                    v8_pair(1)
                    tail_pair(3, u_sb, dpss[3])
                    v8_pair(0)
                    v8_pair(2)
                    v8_pair(3)
                else:
                    korder = (list(range(NCHUNK // 2)) if t % 2 == 0
                              else list(range(NCHUNK // 2 - 1, -1, -1)))
                    for i, k in enumerate(korder):
                        dpss[k] = compute_pair(k, f_sb, u_sb)
                        if i >= 1:
                            kp = korder[i - 1]
                            tail_pair(kp, u_sb, dpss[kp])
                        if i >= 2:
                            v8_pair(korder[i - 2])
                    tail_pair(korder[-1], u_sb, dpss[korder[-1]])
                    v8_pair(korder[-2])
                    v8_pair(korder[-1])

                if _dummy and not last:
                    for _ in range(_dummy):
                        nc.tensor.matmul(dum[:, 0:512], dum_lhsT, dum_rhs,
                                         start=True, stop=True, perf_mode=DR,
                                         tile_position=(0, 0))

            # write back real pixels (frame rows 1..24, cols 1..128)
            a0 = FRAME_OFF + FW + 1
            nc.sync.dma_start(
                out_d[:].rearrange("p (r x) -> p r x", x=W),
                s_sb[:, a0:a0 + SR * FW].rearrange(
                    "p (r x) -> p r x", x=FW)[:, :, 0:W],
            )

    nc.compile()
    return nc


def _prep_weights(w1, b1, w2, b2):
    f8 = np.dtype(dt.np(dt.float8e4))
    bf = np.dtype(dt.np(dt.bfloat16))

    def q8(x):
        return np.asarray(x, np.float32).astype(f8).astype(np.float32)

    A = np.ascontiguousarray(w1[:, 0::3]).astype(np.float32)   # [64, 24]
    Bm = np.ascontiguousarray(w1[:, 1::3]).astype(np.float32)
    Cm = np.ascontiguousarray(w1[:, 2::3]).astype(np.float32)

    qA = q8(A * WS)
    qAlo = q8(A * WS - qA)
    qB = q8(Bm * WS)
    qC = q8(Cm * WS)

    # per-slot (j0, j1) weight matrices [64, 24]; values are already the
    # fp8-representable numbers (exact power-of-2 scalings of qA/qB/qC)
    slots = [
        (qA, qAlo),
        (-qB / 8, qB / 8),
        (-qB / 4, qB / 4),
        (-qB / 8, qB / 8),
        (-qC / 4, qC / 4),
        (qC / 8, qC / 8),
    ]

    tapw8 = np.zeros((128, NSLOT * 256), np.float32)
    for i, (w0, w1s) in enumerate(slots):
        for p in range(2):
            base = 64 * p
            for j, wj in enumerate((w0, w1s)):
                blk = 256 * i + 128 * j
                tapw8[base:base + 24, blk:blk + 64] = wj.T
                tapw8[base + 32:base + 56, blk + 64:blk + 128] = wj.T
    tapw8 = tapw8.astype(f8)

    w2b = np.zeros((128, 2 * 120), np.float32)
    for p in range(2):
        ge, go = 2 * p, 2 * p + 1
        w2b[0:64, 120 * p + 32 * ge:120 * p + 32 * ge + 24] = w2.T
        w2b[64:128, 120 * p + 32 * go:120 * p + 32 * go + 24] = w2.T
    w2b = w2b.astype(bf)

    b2r = np.zeros((128, 1), np.float32)
    b1v = np.zeros((128, 1), np.float32)
    for g in range(4):
        b2r[32 * g:32 * g + 24, 0] = b2
    b1v[0:64, 0] = b1
    b1v[64:128, 0] = b1
    return tapw8, w2b, b2r, b1v


def _prep_state(state):
    """state (B, C, H, W) -> per-core [128, S_FREE] framed slabs."""
    bufs = []
    for core in range(N_CORES):
        b = core // 2
        top = (core % 2) == 0
        r0 = 0 if top else H - SH
        buf = np.zeros((128, S_FREE), np.float32)
        for ch in range(C):
            full = np.zeros((SH + 2, FW), np.float32)
            full[1:SH + 1, 1:W + 1] = state[b, ch, r0:r0 + SH, :]
            if r0 > 0:
                full[0, 1:W + 1] = state[b, ch, r0 - 1, :]
            if r0 + SH < H:
                full[SH + 1, 1:W + 1] = state[b, ch, r0 + SH, :]
            for g in range(4):
                fr = full[g * SR:g * SR + FR, :]
                buf[32 * g + ch, FRAME_OFF:FRAME_OFF + FRAME] = fr.reshape(-1)
        bufs.append(buf)
    return bufs


def _prep_masks(masks):
    """masks (S, B, 1, H, W) -> per-core [S, 128, COMP] fp8 0/1 fire."""
    f8 = np.dtype(dt.np(dt.float8e4))
    S = masks.shape[0]
    bufs = []
    for core in range(N_CORES):
        b = core // 2
        top = (core % 2) == 0
        r0 = 0 if top else H - SH
        mrows = np.zeros((S, SH, FW), np.float32)
        mrows[:, :, 1:W + 1] = (masks[:, b, 0, r0:r0 + SH, :]
                                < FIRE_RATE).astype(np.float32)
        fire = np.zeros((S, 128, COMP), np.float32)
        for g in range(4):
            seg = mrows[:, g * SR:(g + 1) * SR, :].reshape(S, COMP)
            fire[:, 32 * g:32 * g + C, :] = seg[:, None, :]
        bufs.append(fire.astype(f8))
    return bufs


def _prep_all(state, w1, b1, w2, b2, masks):
    tapw8, w2b, b2r, b1v = _prep_weights(w1, b1, w2, b2)
    s_bufs = _prep_state(state)
    f_bufs = _prep_masks(masks)
    in_maps = []
    for core in range(N_CORES):
        in_maps.append({
            "s0": s_bufs[core],
            "fire": f_bufs[core],
            "tapw8": tapw8,
            "w2b": w2b,
            "b2r": b2r,
            "b1v": b1v,
        })
    return in_maps


def _prog_key(masks, b1, b2):
    import os as _os
    steps = masks.shape[0]
    apply_b2 = bool(np.any(b2 != 0))
    apply_b1 = bool(np.any(b1 != 0))
    repeats = int(_os.environ.get("NCA_REPEAT", "1"))
    return ("prog", steps, apply_b2, apply_b1, repeats)


def _get_program(masks, b1, b2):
    key = _prog_key(masks, b1, b2)
    if key not in _cache:
        _cache[key] = _build_program(key[1], key[2], key[3], key[4])
    return _cache[key]


def kernel(state, w1, b1, w2, b2, masks):
    state = np.asarray(state)
    w1, b1 = np.asarray(w1), np.asarray(b1)
    w2, b2 = np.asarray(w2), np.asarray(b2)
    masks = np.asarray(masks)
    nc = _get_program(masks, b1, b2)

    from concourse.bass_utils import run_bass_kernel_spmd

    in_maps = _prep_all(state, w1, b1, w2, b2, masks)

    import os
    trace = bool(os.environ.get("NCA_TRACE"))
    kw = {}
    if trace:
        kw["trace"] = True
        if os.environ.get("NCA_TRACE_DIR"):
            kw["tmpdir"] = os.environ["NCA_TRACE_DIR"]
    res = run_bass_kernel_spmd(nc, in_maps, list(range(N_CORES)), **kw)
    global LAST_EXEC_NS
    LAST_EXEC_NS = res.exec_time_ns

    out = np.zeros((B, C, H, W), np.float32)
    for core in range(N_CORES):
        o = res.results[core]["out"]  # [128, SR*W]
        b = core // 2
        top = (core % 2) == 0
        r0 = 0 if top else H - SH
        own0 = 0 if top else H // 2
        for g in range(4):
            rows = o[32 * g:32 * g + 24].reshape(C, SR, W)
            g0 = r0 + g * SR
            lo = max(g0, own0)
            hi = min(g0 + SR, own0 + H // 2)
            if lo < hi:
                out[b, :, lo:hi, :] = rows[:, lo - g0:hi - g0, :]
    return out
